# revision 14
# baseline (speedup 1.0000x reference)
"""GNN message-passing (GENConv-style, 2 layers x 2 link types) on 8 trn2 cores.

Sharding: partition by destination node range (2500 nodes/core). Each core owns
its nodes' incoming edges for both links/layers. Gather tables (sf, x1) are
kept in GLOBAL SLOT order (core-major, 2560 slots/core incl padding), so table
construction is plain static DMA + one AllGather (no indirect scatters), and
source-feature gathers use dma_gather with global-slot int16 indices.
lin_dst is folded into the first MLP matmul host-side. Edge softmax computed
without max-subtraction as num/den; segment sums via one-hot matmuls with the
one-hot S tiles generated on-chip (iota + is_equal). BN bias applied via the
scalar-engine activation bias operand; leaky_relu via Lrelu activation.
"""

import numpy as np
import ml_dtypes

import concourse.bass as bass
import concourse.mybir as mybir
import concourse.tile as tile
from concourse import bacc
from concourse.bass_utils import run_bass_kernel_spmd
from concourse.masks import make_identity

N_NODES = 20000
FIN = 256
H = 512
H2 = 1024
NCORES = 8
SHARD = N_NODES // NCORES  # 2500
P = 128
NBLK = 20           # slot blocks per core (20*128 = 2560 slots >= 2500)
SLOTS = NBLK * P    # 2560
NCH = SLOTS // H    # 5 slot-chunks of 512
BN_EPS = 1e-5

f32 = mybir.dt.float32
f32r = mybir.dt.float32r
bf16 = mybir.dt.bfloat16
i32 = mybir.dt.int32
i16 = mybir.dt.int16
AF = mybir.ActivationFunctionType
OP = mybir.AluOpType

nbf = np.dtype(ml_dtypes.bfloat16)

_cache = {}


def _pack_lhst(wt):
    """[K, M] -> [128, K//128, M//128, 128] so [:, kt, ch, :] is a lhsT tile."""
    K, M = wt.shape
    return np.ascontiguousarray(
        wt.reshape(K // P, P, M // P, P).transpose(1, 0, 2, 3)
    ).astype(np.float32)


def _bin_pack(d0, d1):
    """Assign SHARD local nodes to NBLK blocks (<=128 nodes each), balancing
    per-link edge load. Returns list of sorted node-id arrays."""
    d_tot = d0 + d1
    order = np.argsort(-d_tot, kind="stable")
    loads = np.zeros(NBLK, dtype=np.int64)
    counts = np.zeros(NBLK, dtype=np.int64)
    blocks = [[] for _ in range(NBLK)]
    for n in order:
        cand = np.where(counts < P)[0]
        b = cand[np.argmin(loads[cand])]
        blocks[b].append(int(n))
        loads[b] += d_tot[n]
        counts[b] += 1
    return [np.array(sorted(b), dtype=np.int64) for b in blocks]


def _host_prep(ei, ea, lens):
    """Build per-core edge-structure inputs. Returns (T, per_core list)."""
    E = ei.shape[1]
    src_all = ei[0].astype(np.int64)
    dst_all = ei[1].astype(np.int64)
    link0 = np.zeros(E, dtype=bool)
    link0[: lens[0]] = True

    per_core = []
    gslot_of_node = np.zeros(N_NODES, dtype=np.int64)
    for c in range(NCORES):
        lo, hi = c * SHARD, (c + 1) * SHARD
        core = {}
        m_core = (dst_all >= lo) & (dst_all < hi)
        dloc_all = dst_all - lo
        d0 = np.bincount(dloc_all[m_core & link0], minlength=SHARD)
        d1 = np.bincount(dloc_all[m_core & ~link0], minlength=SHARD)
        blocks = _bin_pack(d0, d1)

        slot_of_node = np.full(SHARD, -1, dtype=np.int64)
        smap = np.broadcast_to(np.arange(P, dtype=np.int64) + SHARD,
                               (NBLK, P)).copy()
        for b, nodes in enumerate(blocks):
            smap[b, : len(nodes)] = nodes
            slot_of_node[nodes] = b * P + np.arange(len(nodes))
        assert (slot_of_node >= 0).all()
        core["slot_of_node"] = slot_of_node
        core["perm"] = np.minimum(smap.reshape(-1), SHARD)
        gslot_of_node[lo:hi] = c * SLOTS + slot_of_node

        core["links"] = []
        for l in range(2):
            m = m_core & (link0 if l == 0 else ~link0)
            e_idx = np.nonzero(m)[0]
            e_slot = slot_of_node[dst_all[e_idx] - lo]
            e_blk = e_slot // P
            o = np.argsort(e_blk, kind="stable")
            e_idx, e_slot, e_blk = e_idx[o], e_slot[o], e_blk[o]
            cnt = np.bincount(e_blk, minlength=NBLK)
            core["links"].append((e_idx, e_slot, cnt))
        per_core.append(core)

    T = 0
    for core in per_core:
        for (_, _, cnt) in core["links"]:
            T = max(T, int(np.ceil(cnt.max() / P)))

    # chunked-AllGather table layouts: sf table in 4 chunks of 640 slots,
    # x1 table in 5 chunks of 512 slots (chunk-major, then core-major)
    src_node = np.zeros(N_NODES, dtype=np.int64)  # scratch to hold slots
    slot_all = gslot_of_node % SLOTS
    core_all = gslot_of_node // SLOTS
    gsf_of_node = (slot_all // 640) * (NCORES * 640) + core_all * 640 \
        + (slot_all % 640)
    gx1_of_node = (slot_all // 512) * (NCORES * 512) + core_all * 512 \
        + (slot_all % 512)

    NT = NBLK * T
    for core in per_core:
        for l in range(2):
            e_idx, e_slot, cnt = core["links"][l]
            src_n = np.zeros((NBLK, T * P), dtype=np.int64)
            eav = np.zeros((NBLK, T * P), dtype=np.float32)
            esl = np.full((NBLK, T * P), 255.0, dtype=np.float32)
            off = 0
            for b in range(NBLK):
                k = int(cnt[b])
                sl = slice(off, off + k)
                src_n[b, :k] = src_all[e_idx[sl]]
                eav[b, :k] = ea[e_idx[sl], 0]
                esl[b, :k] = (e_slot[sl] - b * P).astype(np.float32)
                off += k
            # dma_gather index order per chunk cc: j = ((bi*T+tt)*128 + p)
            flat = src_n.reshape(NT * P)  # (b, t, p) order already
            for nm, gmap in (("isf", gsf_of_node), ("ix1", gx1_of_node)):
                gi = gmap[flat]
                core[f"{nm}_{l}"] = np.ascontiguousarray(
                    np.tile(gi.reshape(-1, 16).T, (8, 1))).astype(np.int16)
            core[f"ea_{l}"] = np.ascontiguousarray(
                eav.reshape(NT, P).T.astype(nbf))
            core[f"eslot_{l}"] = np.ascontiguousarray(
                esl.reshape(NT, P).T.astype(nbf))
        del core["links"]

    return T, per_core


def _build(T):
    NT = NBLK * T
    NI_CC = 4 * T * P          # gather indices per (l, cchunk)
    NI16_CC = NI_CC // 16      # packed idx columns per cchunk
    GSLOTS = NCORES * SLOTS

    nc = bacc.Bacc("TRN2", target_bir_lowering=False, debug=False,
                   num_devices=NCORES)

    def din(name, shape, dt):
        return nc.dram_tensor(name, shape, dt, kind="ExternalInput")

    xt_sf = din("xt_sf", [P, 2, SLOTS], f32r)     # x^T slot order, sf path
    xt_bf = din("xt_bf", [P, 2, SLOTS], bf16)     # x^T slot order, MLP path
    edge_d = {}
    for l in range(2):
        edge_d[f"isf_{l}"] = din(f"isf_{l}", [P, NT * P // 16], i16)
        edge_d[f"ix1_{l}"] = din(f"ix1_{l}", [P, NT * P // 16], i16)
        edge_d[f"ea_{l}"] = din(f"ea_{l}", [P, NT], bf16)
        edge_d[f"eslot_{l}"] = din(f"eslot_{l}", [P, NT], bf16)
    wst_d = [din(f"wst_{l}", [P, 2, H], f32r) for l in range(2)]
    iota_d = din("iota", [P, P], bf16)
    bnb_d = din("bnb_all", [P, 32], f32)
    wd = {}
    for ll in range(2):
        for l in range(2):
            wd[f"w1t_{ll}_{l}"] = din(f"w1t_{ll}_{l}", [P, 4, 8, P], bf16)
            wd[f"w2t_{ll}_{l}"] = din(f"w2t_{ll}_{l}", [P, 8, 4, P], bf16)
            wd[f"we_{ll}_{l}"] = din(f"we_{ll}_{l}", [P, H], bf16)
    for l in range(2):
        wd[f"xw_0_{l}"] = din(f"xw_0_{l}", [P, 2, 8, P], bf16)   # W1@Wd fold
        wd[f"xw_1_{l}"] = din(f"xw_1_{l}", [P, 4, 8, P], bf16)   # layer1 w1s
    y_ext = nc.dram_tensor("y_out", [4, P, SLOTS], bf16, kind="ExternalOutput")

    sf_bounce = nc.dram_tensor("sf_bounce", [SLOTS, 2 * H], bf16)
    sf_table = nc.dram_tensor("sf_table", [GSLOTS, 2 * H], bf16,
                              addr_space="Shared")
    x1_bounce = nc.dram_tensor("x1_bounce", [SLOTS, H], bf16)
    x1_table = nc.dram_tensor("x1_table", [GSLOTS, H], bf16,
                              addr_space="Shared")

    RG = [list(range(NCORES))]

    with tile.TileContext(nc) as tc:
        with (
            tc.tile_pool(name="const", bufs=1) as cp,
            tc.tile_pool(name="wpool", bufs=1) as wpl,
            tc.tile_pool(name="gp", bufs=2) as gp,
            tc.tile_pool(name="ee", bufs=2) as ee,
            tc.tile_pool(name="sfx", bufs=2) as sfx,
            tc.tile_pool(name="dr", bufs=2) as drp,
            tc.tile_pool(name="agg", bufs=1) as ap_,
            tc.tile_pool(name="hpool", bufs=1) as hp_,
            tc.tile_pool(name="x1p", bufs=1) as x1p,
            tc.tile_pool(name="scratch", bufs=2) as scr,
            tc.tile_pool(name="pseg", bufs=2, space="PSUM") as pseg,
            tc.tile_pool(name="pmlp", bufs=2, space="PSUM") as pmlp,
            tc.tile_pool(name="py", bufs=2, space="PSUM") as py,
        ):
            def load(pool, dram, shape, dt, tag):
                t = pool.tile(shape, dt, tag=tag, name=tag)
                nc.sync.dma_start(out=t[:], in_=dram.ap())
                return t

            xt_b = load(cp, xt_bf, [P, 2, SLOTS], bf16, "xt_b")
            isf_t = [load(cp, edge_d[f"isf_{l}"], [P, NT * P // 16], i16,
                          f"isf{l}") for l in range(2)]
            ix1_t = [load(cp, edge_d[f"ix1_{l}"], [P, NT * P // 16], i16,
                          f"ix1{l}") for l in range(2)]
            ea_t = [load(cp, edge_d[f"ea_{l}"], [P, NT], bf16, f"ea{l}")
                    for l in range(2)]
            esl_t = [load(cp, edge_d[f"eslot_{l}"], [P, NT], bf16, f"esl{l}")
                     for l in range(2)]
            wst_t = [load(cp, wst_d[l], [P, 2, H], f32r, f"wst{l}")
                     for l in range(2)]
            we_t = {}
            for ll in range(2):
                for l in range(2):
                    we_t[(ll, l)] = load(cp, wd[f"we_{ll}_{l}"], [P, H], bf16,
                                         f"we{ll}{l}")
            bnb_t = load(cp, bnb_d, [P, 32], f32, "bnb_all")
            iota_t = load(cp, iota_d, [P, P], bf16, "iota")
            identb = cp.tile([P, P], bf16, tag="identb")
            identf = cp.tile([P, P], f32, tag="identf")
            make_identity(nc, identf[:])
            nc.vector.tensor_copy(out=identb[:], in_=identf[:])

            # ---- sf shards (slot order, both links), chunked AllGathers ----
            for b in range(NBLK):
                xsf = sfx.tile([P, 2, P], f32r, tag="xsf")
                nc.sync.dma_start(out=xsf[:],
                                  in_=xt_sf.ap()[:, :, b * P:(b + 1) * P])
                sfc = sfx.tile([P, 2, H], bf16, tag="sfc")
                for l in range(2):
                    ps = pmlp.tile([P, H], f32, space="PSUM", tag="hp")
                    for kt in range(2):
                        nc.tensor.matmul(
                            out=ps[:],
                            lhsT=xsf[:, kt, :],
                            rhs=wst_t[l][:, kt, :],
                            start=(kt == 0), stop=(kt == 1))
                    nc.scalar.activation(out=sfc[:, l, :], in_=ps[:],
                                         func=AF.Copy)
                nc.sync.dma_start(out=sf_bounce.ap()[b * P:(b + 1) * P, :],
                                  in_=sfc[:])
                if b % 5 == 4:   # 4 chunks of 5 blocks (640 slots)
                    k = b // 5
                    nc.gpsimd.collective_compute(
                        "AllGather", OP.bypass, replica_groups=RG,
                        ins=[sf_bounce.ap()[k * 640:(k + 1) * 640, :].opt()],
                        outs=[sf_table.ap()[k * 5120:(k + 1) * 5120, :].opt()])

            x1T = x1p.tile([P, 4, SLOTS], bf16, tag="x1T")

            def layer(ll):
                w1t = {}
                w2t = {}
                xw = {}
                for l in range(2):
                    w1t[l] = load(wpl, wd[f"w1t_{ll}_{l}"], [P, 4, 8, P],
                                  bf16, f"w1t{l}")
                    w2t[l] = load(wpl, wd[f"w2t_{ll}_{l}"], [P, 8, 4, P],
                                  bf16, f"w2t{l}")
                    xw[l] = wpl.tile([P, 4, 8, P], bf16, tag=f"xw{l}",
                                     name=f"xw{l}")
                    if ll == 0:
                        nc.sync.dma_start(out=xw[l][:, 0:2],
                                          in_=wd[f"xw_0_{l}"].ap())
                    else:
                        nc.sync.dma_start(out=xw[l][:],
                                          in_=wd[f"xw_1_{l}"].ap())
                for cchunk in range(NCH):
                    c0 = cchunk * H
                    aggT = {}
                    for l in range(2):
                        g = gp.tile([P, 4 * T, H], bf16, tag="g", name="g")
                        idx_t = isf_t if ll == 0 else ix1_t
                        nc.gpsimd.dma_gather(
                            out_ap=g[:],
                            in_ap=(sf_table.ap()[:, l * H:(l + 1) * H]
                                   if ll == 0 else x1_table.ap()[:, 0:H]),
                            idxs_ap=idx_t[l][:, cchunk * NI16_CC:
                                             (cchunk + 1) * NI16_CC],
                            num_idxs=NI_CC,
                            num_idxs_reg=NI_CC,
                            elem_size=H,
                            elem_step=(2 * H if ll == 0 else H),
                            single_packet=False,
                        )
                        aggT[l] = ap_.tile([P, 4, H], bf16, tag=f"aggT{l}",
                                           name=f"aggT{l}")
                        for bi in range(4):
                            b = cchunk * 4 + bi
                            # edge math, batched across the T tiles of block b
                            z = ee.tile([P, T * H], bf16, tag="z")
                            nc.vector.tensor_tensor(
                                out=z[:].rearrange("p (t h) -> p t h", t=T),
                                in0=we_t[(ll, l)][:].rearrange(
                                    "p (a h) -> p a h", a=1
                                ).broadcast_to([P, T, H]),
                                in1=ea_t[l][:, b * T:(b + 1) * T].rearrange(
                                    "p (t o) -> p t o", o=1
                                ).broadcast_to([P, T, H]),
                                op=OP.mult)
                            nc.vector.tensor_tensor(
                                out=z[:], in0=z[:],
                                in1=g[:, bi * T:(bi + 1) * T, :].rearrange(
                                    "p a b -> p (a b)"),
                                op=OP.add)
                            nc.vector.tensor_scalar(
                                out=z[:], in0=z[:], scalar1=0.0,
                                scalar2=None, op0=OP.max)
                            pe = ee.tile([P, T * H], bf16, tag="pe")
                            half = (T * H) // 2
                            nc.scalar.activation(out=pe[:, :half],
                                                 in_=z[:, :half], func=AF.Exp)
                            nc.scalar.activation(out=pe[:, half:],
                                                 in_=z[:, half:], func=AF.Exp)
                            qe = ee.tile([P, T * H], bf16, tag="qe")
                            nc.vector.tensor_tensor(
                                out=qe[:], in0=z[:], in1=pe[:], op=OP.mult)
                            den = pseg.tile([P, H], f32, space="PSUM",
                                            tag="den")
                            num = pseg.tile([P, H], f32, space="PSUM",
                                            tag="num")
                            s_all = ee.tile([P, T * P], bf16, tag="S")
                            nc.vector.tensor_tensor(
                                out=s_all[:].rearrange("p (t q) -> p t q",
                                                       t=T),
                                in0=iota_t[:].rearrange(
                                    "p (a q) -> p a q", a=1
                                ).broadcast_to([P, T, P]),
                                in1=esl_t[l][:, b * T:(b + 1) * T].rearrange(
                                    "p (t o) -> p t o", o=1
                                ).broadcast_to([P, T, P]),
                                op=OP.is_equal)
                            for tt in range(T):
                                s_tile = s_all[:, tt * P:(tt + 1) * P]
                                for j in range(4):
                                    nc.tensor.matmul(
                                        out=den[:, j * P:(j + 1) * P],
                                        lhsT=pe[:, tt * H + j * P:
                                                tt * H + (j + 1) * P],
                                        rhs=s_tile,
                                        start=(tt == 0 and j == 0),
                                        stop=(tt == T - 1 and j == 3),
                                        skip_group_check=True)
                                for j in range(4):
                                    nc.tensor.matmul(
                                        out=num[:, j * P:(j + 1) * P],
                                        lhsT=qe[:, tt * H + j * P:
                                                tt * H + (j + 1) * P],
                                        rhs=s_tile,
                                        start=(tt == 0 and j == 0),
                                        stop=(tt == T - 1 and j == 3),
                                        skip_group_check=True)
                            dens = drp.tile([P, H], f32, tag="dens")
                            nc.vector.tensor_scalar(
                                out=dens[:], in0=den[:], scalar1=1e-30,
                                scalar2=None, op0=OP.max)
                            rden = drp.tile([P, H], f32, tag="rden")
                            nc.vector.reciprocal_approx_fast(
                                out=rden[:], in_=dens[:])
                            nc.vector.tensor_tensor(
                                out=aggT[l][:, :, bi * P:(bi + 1) * P],
                                in0=num[:].rearrange("p (a b) -> p a b", a=4),
                                in1=rden[:].rearrange("p (a b) -> p a b", a=4),
                                op=OP.mult)
                    hs = {}
                    for l in range(2):
                        hs[l] = hp_.tile([P, 8, H], bf16, tag=f"h{l}",
                                         name=f"h{l}")
                        for ch in range(8):
                            hp = pmlp.tile([P, H], f32, space="PSUM", tag="hp")
                            if ll == 0:
                                for kt in range(2):
                                    nc.tensor.matmul(
                                        out=hp[:],
                                        lhsT=xw[l][:, kt, ch, :],
                                        rhs=xt_b[:, kt, c0:c0 + H],
                                        start=(kt == 0), stop=False)
                            else:
                                for kt in range(4):
                                    nc.tensor.matmul(
                                        out=hp[:],
                                        lhsT=xw[l][:, kt, ch, :],
                                        rhs=x1T[:, kt, c0:c0 + H],
                                        start=(kt == 0), stop=False)
                            for kt in range(4):
                                nc.tensor.matmul(
                                    out=hp[:],
                                    lhsT=w1t[l][:, kt, ch, :],
                                    rhs=aggT[l][:, kt, :],
                                    start=False, stop=(kt == 3))
                            nc.scalar.activation(
                                out=hs[l][:, ch, :], in_=hp[:], func=AF.Relu,
                                bias=bnb_t[:, (ll * 2 + l) * 8 + ch:
                                           (ll * 2 + l) * 8 + ch + 1])
                    for ch3 in range(4):
                        yp = py.tile([P, H], f32, space="PSUM", tag="yp")
                        for l in range(2):
                            for kt in range(8):
                                nc.tensor.matmul(
                                    out=yp[:],
                                    lhsT=w2t[l][:, kt, ch3, :],
                                    rhs=hs[l][:, kt, :],
                                    start=(l == 0 and kt == 0),
                                    stop=(l == 1 and kt == 7))
                        if ll == 0:
                            nc.scalar.activation(
                                out=x1T[:, ch3, c0:c0 + H], in_=yp[:],
                                func=AF.Lrelu, alpha=0.01)
                        else:
                            ysb = scr.tile([P, H], bf16, tag="ysb")
                            nc.scalar.activation(out=ysb[:], in_=yp[:],
                                                 func=AF.Copy)
                            nc.sync.dma_start(
                                out=y_ext.ap()[ch3, :, c0:c0 + H],
                                in_=ysb[:])
                    if ll == 0:
                        for bi in range(4):
                            b = cchunk * 4 + bi
                            pt = pmlp.tile([P, H], bf16, space="PSUM",
                                           tag="hp")
                            for j in range(4):
                                nc.tensor.transpose(
                                    out=pt[:, j * P:(j + 1) * P],
                                    in_=x1T[:, j, c0 + bi * P:
                                            c0 + (bi + 1) * P],
                                    identity=identb[:])
                            rows = scr.tile([P, H], bf16, tag="rows")
                            nc.scalar.activation(out=rows[:], in_=pt[:],
                                                 func=AF.Copy)
                            nc.sync.dma_start(
                                out=x1_bounce.ap()[b * P:(b + 1) * P, :],
                                in_=rows[:])
                        nc.gpsimd.collective_compute(
                            "AllGather", OP.bypass, replica_groups=RG,
                            ins=[x1_bounce.ap()[c0:c0 + H, :].opt()],
                            outs=[x1_table.ap()[cchunk * 4096:
                                                (cchunk + 1) * 4096, :].opt()])

            layer(0)
            layer(1)

    nc.compile()
    return nc


def _prep_weights(inputs):
    out = {}
    bnb_cols = np.zeros((P, 32), np.float32)
    for ll in range(2):
        pre = "l0" if ll == 0 else "l1"
        for l in range(2):
            w1 = np.asarray(inputs[f"{pre}_w1"][l], np.float32)   # [1024, 512]
            w2 = np.asarray(inputs[f"{pre}_w2"][l], np.float32)   # [512, 1024]
            g = np.asarray(inputs[f"{pre}_g"][l], np.float32)
            b = np.asarray(inputs[f"{pre}_b"][l], np.float32)
            m = np.asarray(inputs[f"{pre}_m"][l], np.float32)
            v = np.asarray(inputs[f"{pre}_v"][l], np.float32)
            we = np.asarray(inputs[f"{pre}_edge"][l], np.float32)[:, 0]
            s = g / np.sqrt(v + BN_EPS)
            bb = b - m * s
            w1s = s[:, None] * w1
            out[f"w1t_{ll}_{l}"] = _pack_lhst(w1s.T).astype(nbf)
            out[f"w2t_{ll}_{l}"] = _pack_lhst(w2.T).astype(nbf)
            out[f"we_{ll}_{l}"] = np.ascontiguousarray(
                np.broadcast_to(we.astype(nbf), (P, H)))
            bnb_cols[:, (ll * 2 + l) * 8:(ll * 2 + l) * 8 + 8] = \
                bb.reshape(8, P).T
            if ll == 0:
                ws = np.asarray(inputs["l0_src"][l], np.float32)
                wdm = np.asarray(inputs["l0_dst"][l], np.float32)
                wf = s[:, None] * (w1 @ wdm)                      # [1024, 256]
                out[f"wst_{l}"] = np.ascontiguousarray(
                    ws.T.reshape(2, P, H).transpose(1, 0, 2))
                out[f"xw_0_{l}"] = _pack_lhst(wf.T).astype(nbf)
            else:
                out[f"xw_1_{l}"] = _pack_lhst(w1s.T).astype(nbf)
    out["bnb_all"] = bnb_cols
    out["iota"] = np.ascontiguousarray(
        np.broadcast_to(np.arange(P, dtype=np.float32), (P, P))).astype(nbf)
    return out


def kernel(**inputs):
    x = np.asarray(inputs["x"], np.float32)
    ei = np.asarray(inputs["ei_flat"], np.int32)
    ea = np.asarray(inputs["ea_flat"], np.float32)
    lens = (int(inputs["len0"]), int(inputs["len1"]))

    T, per_core = _host_prep(ei, ea, lens)
    wshared = _prep_weights(inputs)

    if T not in _cache:
        _cache[T] = _build(T)
    nc = _cache[T]

    in_maps = []
    for c in range(NCORES):
        core = per_core[c]
        lo = c * SHARD
        xs = x[lo:lo + SHARD]
        xs_pad = np.vstack([xs, np.zeros((1, FIN), np.float32)])
        xt2 = np.ascontiguousarray(xs_pad[core["perm"]].T)        # [256, SLOTS]
        xt_slt = np.ascontiguousarray(
            xt2.reshape(2, P, SLOTS).transpose(1, 0, 2))
        im = dict(xt_sf=xt_slt, xt_bf=xt_slt.astype(nbf))
        for l in range(2):
            im[f"isf_{l}"] = core[f"isf_{l}"]
            im[f"ix1_{l}"] = core[f"ix1_{l}"]
            im[f"ea_{l}"] = core[f"ea_{l}"]
            im[f"eslot_{l}"] = core[f"eslot_{l}"]
        im.update(wshared)
        in_maps.append(im)

    res = run_bass_kernel_spmd(nc, in_maps, core_ids=list(range(NCORES)))
    globals()["LAST_RESULT"] = res
    out = np.empty((N_NODES, H), np.float32)
    for c in range(NCORES):
        y = res.results[c]["y_out"].astype(np.float32)   # [4, 128, SLOTS]
        y_slots = y.reshape(H, SLOTS).T                  # [SLOTS, H]
        out[c * SHARD:(c + 1) * SHARD] = y_slots[per_core[c]["slot_of_node"]]
    return np.ascontiguousarray(out)


# revision 29
# speedup vs baseline: 1.1652x; 1.1652x over previous
"""GNN message-passing (GENConv-style, 2 layers x 2 link types) on 8 trn2 cores.

Sharding: partition by destination node range (2500 nodes/core). Each core owns
its nodes' incoming edges for both links/layers. Gather tables (sf, x1) are
kept in GLOBAL SLOT order (core-major, 2560 slots/core incl padding), so table
construction is plain static DMA + one AllGather (no indirect scatters), and
source-feature gathers use dma_gather with global-slot int16 indices.
lin_dst is folded into the first MLP matmul host-side. Edge softmax computed
without max-subtraction as num/den; segment sums via one-hot matmuls with the
one-hot S tiles generated on-chip (iota + is_equal). BN bias applied via the
scalar-engine activation bias operand; leaky_relu via Lrelu activation.
"""

import numpy as np
import ml_dtypes

import concourse.bass as bass
import concourse.mybir as mybir
import concourse.tile as tile
from concourse import bacc
from concourse.bass_utils import run_bass_kernel_spmd
from concourse.masks import make_identity

N_NODES = 20000
FIN = 256
H = 512
H2 = 1024
NCORES = 8
SHARD = N_NODES // NCORES  # 2500
P = 128
NBLK = 20           # slot blocks per core (20*128 = 2560 slots >= 2500)
SLOTS = NBLK * P    # 2560
NCH = SLOTS // H    # 5 slot-chunks of 512
BN_EPS = 1e-5

f32 = mybir.dt.float32
f32r = mybir.dt.float32r
bf16 = mybir.dt.bfloat16
i32 = mybir.dt.int32
i16 = mybir.dt.int16
AF = mybir.ActivationFunctionType
OP = mybir.AluOpType

nbf = np.dtype(ml_dtypes.bfloat16)

_cache = {}


def _pack_lhst(wt):
    """[K, M] -> [128, K//128, M//128, 128] so [:, kt, ch, :] is a lhsT tile."""
    K, M = wt.shape
    return np.ascontiguousarray(
        wt.reshape(K // P, P, M // P, P).transpose(1, 0, 2, 3)
    ).astype(np.float32)


def _bin_pack(d0, d1):
    """Assign SHARD local nodes to NBLK blocks (<=128 nodes each), balancing
    per-link edge load. Returns list of sorted node-id arrays."""
    d_tot = d0 + d1
    order = np.argsort(-d_tot, kind="stable")
    loads = np.zeros(NBLK, dtype=np.int64)
    counts = np.zeros(NBLK, dtype=np.int64)
    blocks = [[] for _ in range(NBLK)]
    for n in order:
        cand = np.where(counts < P)[0]
        b = cand[np.argmin(loads[cand])]
        blocks[b].append(int(n))
        loads[b] += d_tot[n]
        counts[b] += 1
    return [np.array(sorted(b), dtype=np.int64) for b in blocks]


def _host_prep(ei, ea, lens):
    """Build per-core edge-structure inputs. Returns (T, per_core list)."""
    E = ei.shape[1]
    src_all = ei[0].astype(np.int64)
    dst_all = ei[1].astype(np.int64)
    link0 = np.zeros(E, dtype=bool)
    link0[: lens[0]] = True

    per_core = []
    gslot_of_node = np.zeros(N_NODES, dtype=np.int64)
    for c in range(NCORES):
        lo, hi = c * SHARD, (c + 1) * SHARD
        core = {}
        m_core = (dst_all >= lo) & (dst_all < hi)
        dloc_all = dst_all - lo
        d0 = np.bincount(dloc_all[m_core & link0], minlength=SHARD)
        d1 = np.bincount(dloc_all[m_core & ~link0], minlength=SHARD)
        blocks = _bin_pack(d0, d1)

        slot_of_node = np.full(SHARD, -1, dtype=np.int64)
        smap = np.broadcast_to(np.arange(P, dtype=np.int64) + SHARD,
                               (NBLK, P)).copy()
        for b, nodes in enumerate(blocks):
            smap[b, : len(nodes)] = nodes
            slot_of_node[nodes] = b * P + np.arange(len(nodes))
        assert (slot_of_node >= 0).all()
        core["slot_of_node"] = slot_of_node
        core["perm"] = np.minimum(smap.reshape(-1), SHARD)
        gslot_of_node[lo:hi] = c * SLOTS + slot_of_node

        core["links"] = []
        for l in range(2):
            m = m_core & (link0 if l == 0 else ~link0)
            e_idx = np.nonzero(m)[0]
            e_slot = slot_of_node[dst_all[e_idx] - lo]
            e_blk = e_slot // P
            o = np.argsort(e_blk, kind="stable")
            e_idx, e_slot, e_blk = e_idx[o], e_slot[o], e_blk[o]
            cnt = np.bincount(e_blk, minlength=NBLK)
            core["links"].append((e_idx, e_slot, cnt))
        per_core.append(core)

    T = 0
    for core in per_core:
        for (_, _, cnt) in core["links"]:
            T = max(T, int(np.ceil(cnt.max() / P)))

    # chunked-AllGather table layouts: sf table in 4 chunks of 640 slots,
    # x1 table in 5 chunks of 512 slots (chunk-major, then core-major)
    src_node = np.zeros(N_NODES, dtype=np.int64)  # scratch to hold slots
    slot_all = gslot_of_node % SLOTS
    core_all = gslot_of_node // SLOTS
    gsf_of_node = (slot_all // 640) * (NCORES * 640) + core_all * 640 \
        + (slot_all % 640)
    gx1_of_node = (slot_all // 512) * (NCORES * 512) + core_all * 512 \
        + (slot_all % 512)

    NT = NBLK * T
    for core in per_core:
        for l in range(2):
            e_idx, e_slot, cnt = core["links"][l]
            src_n = np.zeros((NBLK, T * P), dtype=np.int64)
            eav = np.zeros((NBLK, T * P), dtype=np.float32)
            esl = np.full((NBLK, T * P), 255.0, dtype=np.float32)
            off = 0
            for b in range(NBLK):
                k = int(cnt[b])
                sl = slice(off, off + k)
                src_n[b, :k] = src_all[e_idx[sl]]
                eav[b, :k] = ea[e_idx[sl], 0]
                esl[b, :k] = (e_slot[sl] - b * P).astype(np.float32)
                off += k
            # dma_gather index order per chunk cc: j = ((bi*T+tt)*128 + p)
            flat = src_n.reshape(NT * P)  # (b, t, p) order already
            for nm, gmap in (("isf", gsf_of_node), ("ix1", gx1_of_node)):
                gi = gmap[flat]
                core[f"{nm}_{l}"] = np.ascontiguousarray(
                    np.tile(gi.reshape(-1, 16).T, (8, 1))).astype(np.int16)
            core[f"ea_{l}"] = np.ascontiguousarray(
                eav.reshape(NT, P).T).astype(np.float32)
            core[f"eslot_{l}"] = np.ascontiguousarray(
                esl.reshape(NT, P).T.astype(nbf))
        del core["links"]

    return T, per_core


def _build(T):
    NT = NBLK * T
    NI_CC = 4 * T * P          # gather indices per (l, cchunk)
    NI16_CC = NI_CC // 16      # packed idx columns per cchunk
    GSLOTS = NCORES * SLOTS

    nc = bacc.Bacc("TRN2", target_bir_lowering=False, debug=False,
                   num_devices=NCORES)

    def din(name, shape, dt):
        return nc.dram_tensor(name, shape, dt, kind="ExternalInput")

    xt_sf = din("xt_sf", [P, 2, SLOTS], f32r)     # x^T slot order, sf path
    xt_bf = din("xt_bf", [P, 2, SLOTS], bf16)     # x^T slot order, MLP path
    edge_d = {}
    for l in range(2):
        edge_d[f"isf_{l}"] = din(f"isf_{l}", [P, NT * P // 16], i16)
        edge_d[f"ix1_{l}"] = din(f"ix1_{l}", [P, NT * P // 16], i16)
        edge_d[f"ea_{l}"] = din(f"ea_{l}", [P, NT], f32)
        edge_d[f"eslot_{l}"] = din(f"eslot_{l}", [P, NT], bf16)
    wst_d = [din(f"wst_{l}", [P, 2, H], f32r) for l in range(2)]
    iota_d = din("iota", [P, P], bf16)
    bnb_d = din("bnb_all", [P, 32], f32)
    wd = {}
    for ll in range(2):
        for l in range(2):
            wd[f"w1t_{ll}_{l}"] = din(f"w1t_{ll}_{l}", [P, 4, 8, P], bf16)
            wd[f"w2t_{ll}_{l}"] = din(f"w2t_{ll}_{l}", [P, 8, 4, P], bf16)
            wd[f"we_{ll}_{l}"] = din(f"we_{ll}_{l}", [P, H], bf16)
    for l in range(2):
        wd[f"xw_0_{l}"] = din(f"xw_0_{l}", [P, 2, 8, P], bf16)   # W1@Wd fold
        wd[f"xw_1_{l}"] = din(f"xw_1_{l}", [P, 4, 8, P], bf16)   # layer1 w1s
    y_ext = nc.dram_tensor("y_out", [4, P, SLOTS], bf16, kind="ExternalOutput")

    sf_bounce = nc.dram_tensor("sf_bounce", [SLOTS, 2 * H], bf16)
    sf_table = nc.dram_tensor("sf_table", [GSLOTS, 2 * H], bf16,
                              addr_space="Shared")
    x1_bounce = nc.dram_tensor("x1_bounce", [SLOTS, H], bf16)
    x1_table = nc.dram_tensor("x1_table", [GSLOTS, H], bf16,
                              addr_space="Shared")

    RG = [list(range(NCORES))]

    with tile.TileContext(nc) as tc:
        with (
            tc.tile_pool(name="const", bufs=1) as cp,
            tc.tile_pool(name="wpool", bufs=1) as wpl,
            tc.tile_pool(name="gp", bufs=3) as gp,
            tc.tile_pool(name="ee", bufs=2) as ee,
            tc.tile_pool(name="sfx", bufs=1) as sfx,
            tc.tile_pool(name="xs", bufs=2) as xsp,
            tc.tile_pool(name="dr", bufs=1) as drp,
            tc.tile_pool(name="agg", bufs=2) as ap_,
            tc.tile_pool(name="hpool", bufs=1) as hp_,
            tc.tile_pool(name="x1p", bufs=1) as x1p,
            tc.tile_pool(name="scratch", bufs=2) as scr,
            tc.tile_pool(name="pseg", bufs=2, space="PSUM") as pseg,
            tc.tile_pool(name="pmlp", bufs=2, space="PSUM") as pmlp,
            tc.tile_pool(name="py", bufs=2, space="PSUM") as py,
        ):
            def load(pool, dram, shape, dt, tag):
                t = pool.tile(shape, dt, tag=tag, name=tag)
                nc.sync.dma_start(out=t[:], in_=dram.ap())
                return t

            isf_t = [load(cp, edge_d[f"isf_{l}"], [P, NT * P // 16], i16,
                          f"isf{l}") for l in range(2)]
            ix1_t = [load(cp, edge_d[f"ix1_{l}"], [P, NT * P // 16], i16,
                          f"ix1{l}") for l in range(2)]
            ea_t = [load(cp, edge_d[f"ea_{l}"], [P, NT], f32, f"ea{l}")
                    for l in range(2)]
            esl_t = [load(cp, edge_d[f"eslot_{l}"], [P, NT], bf16, f"esl{l}")
                     for l in range(2)]
            wst_t = [load(cp, wst_d[l], [P, 2, H], f32r, f"wst{l}")
                     for l in range(2)]
            we_t = {}
            for ll in range(2):
                for l in range(2):
                    we_t[(ll, l)] = load(cp, wd[f"we_{ll}_{l}"], [P, H], bf16,
                                         f"we{ll}{l}")
            bnb_t = load(cp, bnb_d, [P, 32], f32, "bnb_all")
            iota_t = load(cp, iota_d, [P, P], bf16, "iota")
            identb = cp.tile([P, P], bf16, tag="identb")
            make_identity(nc, identb[:])

            # ---- sf shards (slot order, both links), chunked AllGathers ----
            for b in range(NBLK):
                xsf = sfx.tile([P, 2, P], f32r, tag="xsf")
                nc.sync.dma_start(out=xsf[:],
                                  in_=xt_sf.ap()[:, :, b * P:(b + 1) * P])
                sfc = sfx.tile([P, 2, H], bf16, tag="sfc")
                for l in range(2):
                    ps = pmlp.tile([P, H], f32, space="PSUM", tag="hp")
                    for kt in range(2):
                        nc.tensor.matmul(
                            out=ps[:],
                            lhsT=xsf[:, kt, :],
                            rhs=wst_t[l][:, kt, :],
                            start=(kt == 0), stop=(kt == 1))
                    nc.scalar.activation(out=sfc[:, l, :], in_=ps[:],
                                         func=AF.Copy)
                nc.sync.dma_start(out=sf_bounce.ap()[b * P:(b + 1) * P, :],
                                  in_=sfc[:])
                if b % 5 == 4:   # 4 chunks of 5 blocks (640 slots)
                    k = b // 5
                    nc.gpsimd.collective_compute(
                        "AllGather", OP.bypass, replica_groups=RG,
                        ins=[sf_bounce.ap()[k * 640:(k + 1) * 640, :].opt()],
                        outs=[sf_table.ap()[k * 5120:(k + 1) * 5120, :].opt()])

            x1T = x1p.tile([P, 4, SLOTS], bf16, tag="x1T")

            def layer(ll):
                w1t = {}
                w2t = {}
                xw = {}
                for l in range(2):
                    w1t[l] = load(wpl, wd[f"w1t_{ll}_{l}"], [P, 4, 8, P],
                                  bf16, f"w1t{l}")
                    w2t[l] = load(wpl, wd[f"w2t_{ll}_{l}"], [P, 8, 4, P],
                                  bf16, f"w2t{l}")
                    xw[l] = wpl.tile([P, 4, 8, P], bf16, tag=f"xw{l}",
                                     name=f"xw{l}")
                    if ll == 0:
                        nc.sync.dma_start(out=xw[l][:, 0:2],
                                          in_=wd[f"xw_0_{l}"].ap())
                    else:
                        nc.sync.dma_start(out=xw[l][:],
                                          in_=wd[f"xw_1_{l}"].ap())
                for cchunk in range(NCH):
                    c0 = cchunk * H
                    if ll == 0:
                        xck = xsp.tile([P, 2, H], bf16, tag="xck")
                        nc.sync.dma_start(out=xck[:],
                                          in_=xt_bf.ap()[:, :, c0:c0 + H])
                    aggT = {}
                    for l in range(2):
                        idx_t = isf_t if ll == 0 else ix1_t
                        ghalf = []
                        for gh in range(2):
                            g = gp.tile([P, 2 * T, H], bf16, tag="g", name="g")
                            nc.gpsimd.dma_gather(
                                out_ap=g[:],
                                in_ap=(sf_table.ap()[:, l * H:(l + 1) * H]
                                       if ll == 0 else x1_table.ap()[:, 0:H]),
                                idxs_ap=idx_t[l][
                                    :, cchunk * NI16_CC + gh * NI16_CC // 2:
                                    cchunk * NI16_CC + (gh + 1) * NI16_CC // 2],
                                num_idxs=NI_CC // 2,
                                num_idxs_reg=NI_CC // 2,
                                elem_size=H,
                                elem_step=(2 * H if ll == 0 else H),
                            )
                            ghalf.append(g)
                        aggT[l] = ap_.tile([P, 4, H], bf16, tag=f"aggT{l}",
                                           name=f"aggT{l}")
                        dsb = drp.tile([P, 4, H], f32, tag="dsb")
                        nsb = drp.tile([P, 4, H], f32, tag="nsb")
                        for bi in range(4):
                            b = cchunk * 4 + bi
                            g = ghalf[bi // 2]
                            # edge math, batched across the T tiles of block b
                            z = ee.tile([P, T * H], bf16, tag="z")
                            for tt in range(T):
                                t_idx = b * T + tt
                                nc.vector.tensor_scalar(
                                    out=z[:, tt * H:(tt + 1) * H],
                                    in0=we_t[(ll, l)][:],
                                    scalar1=ea_t[l][:, t_idx:t_idx + 1],
                                    scalar2=None, op0=OP.mult)
                            nc.vector.tensor_tensor(
                                out=z[:], in0=z[:],
                                in1=g[:, (bi % 2) * T:(bi % 2 + 1) * T,
                                      :].rearrange("p a b -> p (a b)"),
                                op=OP.add)
                            nc.vector.tensor_scalar(
                                out=z[:], in0=z[:], scalar1=0.0,
                                scalar2=None, op0=OP.max)
                            pe = ee.tile([P, T * H], bf16, tag="pe")
                            nc.scalar.activation(out=pe[:], in_=z[:],
                                                 func=AF.Exp)
                            qe = ee.tile([P, T * H], bf16, tag="qe")
                            nc.vector.tensor_tensor(
                                out=qe[:], in0=z[:], in1=pe[:], op=OP.mult)
                            den = pseg.tile([P, H], f32, space="PSUM",
                                            tag="den")
                            num = pseg.tile([P, H], f32, space="PSUM",
                                            tag="num")
                            s_all = ee.tile([P, T * P], bf16, tag="S")
                            nc.vector.tensor_tensor(
                                out=s_all[:].rearrange("p (t q) -> p t q",
                                                       t=T),
                                in0=iota_t[:].rearrange(
                                    "p (a q) -> p a q", a=1
                                ).broadcast_to([P, T, P]),
                                in1=esl_t[l][:, b * T:(b + 1) * T].rearrange(
                                    "p (t o) -> p t o", o=1
                                ).broadcast_to([P, T, P]),
                                op=OP.is_equal)
                            for tt in range(T):
                                s_tile = s_all[:, tt * P:(tt + 1) * P]
                                for j in range(4):
                                    nc.tensor.matmul(
                                        out=den[:, j * P:(j + 1) * P],
                                        lhsT=pe[:, tt * H + j * P:
                                                tt * H + (j + 1) * P],
                                        rhs=s_tile,
                                        start=(tt == 0 and j == 0),
                                        stop=(tt == T - 1 and j == 3),
                                        skip_group_check=True)
                                for j in range(4):
                                    nc.tensor.matmul(
                                        out=num[:, j * P:(j + 1) * P],
                                        lhsT=qe[:, tt * H + j * P:
                                                tt * H + (j + 1) * P],
                                        rhs=s_tile,
                                        start=(tt == 0 and j == 0),
                                        stop=(tt == T - 1 and j == 3),
                                        skip_group_check=True)
                            nc.scalar.activation(out=dsb[:, bi, :],
                                                 in_=den[:], func=AF.Copy,
                                                 bias=1e-30)
                            nc.scalar.activation(out=nsb[:, bi, :],
                                                 in_=num[:], func=AF.Copy)
                        # batched softmax division for the whole cchunk
                        rden = drp.tile([P, 4, H], f32, tag="rden")
                        nc.vector.reciprocal_approx_fast(
                            out=rden[:].rearrange("p a b -> p (a b)"),
                            in_=dsb[:].rearrange("p a b -> p (a b)"))
                        for bi in range(4):
                            nc.vector.tensor_tensor(
                                out=aggT[l][:, :, bi * P:(bi + 1) * P],
                                in0=nsb[:, bi, :].rearrange(
                                    "p (a b) -> p a b", a=4),
                                in1=rden[:, bi, :].rearrange(
                                    "p (a b) -> p a b", a=4),
                                op=OP.mult)
                    hs = {}
                    for l in range(2):
                        hs[l] = hp_.tile([P, 8, H], bf16, tag=f"h{l}",
                                         name=f"h{l}")
                        for ch in range(8):
                            hp = pmlp.tile([P, H], f32, space="PSUM", tag="hp")
                            if ll == 0:
                                for kt in range(2):
                                    nc.tensor.matmul(
                                        out=hp[:],
                                        lhsT=xw[l][:, kt, ch, :],
                                        rhs=xck[:, kt, :],
                                        start=(kt == 0), stop=False)
                            else:
                                for kt in range(4):
                                    nc.tensor.matmul(
                                        out=hp[:],
                                        lhsT=xw[l][:, kt, ch, :],
                                        rhs=x1T[:, kt, c0:c0 + H],
                                        start=(kt == 0), stop=False)
                            for kt in range(4):
                                nc.tensor.matmul(
                                    out=hp[:],
                                    lhsT=w1t[l][:, kt, ch, :],
                                    rhs=aggT[l][:, kt, :],
                                    start=False, stop=(kt == 3))
                            nc.scalar.activation(
                                out=hs[l][:, ch, :], in_=hp[:], func=AF.Relu,
                                bias=bnb_t[:, (ll * 2 + l) * 8 + ch:
                                           (ll * 2 + l) * 8 + ch + 1])
                    for ch3 in range(4):
                        yp = py.tile([P, H], f32, space="PSUM", tag="yp")
                        for l in range(2):
                            for kt in range(8):
                                nc.tensor.matmul(
                                    out=yp[:],
                                    lhsT=w2t[l][:, kt, ch3, :],
                                    rhs=hs[l][:, kt, :],
                                    start=(l == 0 and kt == 0),
                                    stop=(l == 1 and kt == 7))
                        if ll == 0:
                            nc.scalar.activation(
                                out=x1T[:, ch3, c0:c0 + H], in_=yp[:],
                                func=AF.Lrelu, alpha=0.01)
                        else:
                            ysb = scr.tile([P, H], bf16, tag="ysb")
                            nc.scalar.activation(out=ysb[:], in_=yp[:],
                                                 func=AF.Copy)
                            nc.sync.dma_start(
                                out=y_ext.ap()[ch3, :, c0:c0 + H],
                                in_=ysb[:])
                    if ll == 0:
                        for bi in range(4):
                            b = cchunk * 4 + bi
                            pt = pmlp.tile([P, H], bf16, space="PSUM",
                                           tag="hp")
                            for j in range(4):
                                nc.tensor.transpose(
                                    out=pt[:, j * P:(j + 1) * P],
                                    in_=x1T[:, j, c0 + bi * P:
                                            c0 + (bi + 1) * P],
                                    identity=identb[:])
                            rows = scr.tile([P, H], bf16, tag="rows")
                            nc.scalar.activation(out=rows[:], in_=pt[:],
                                                 func=AF.Copy)
                            nc.sync.dma_start(
                                out=x1_bounce.ap()[b * P:(b + 1) * P, :],
                                in_=rows[:])
                        nc.gpsimd.collective_compute(
                            "AllGather", OP.bypass, replica_groups=RG,
                            ins=[x1_bounce.ap()[c0:c0 + H, :].opt()],
                            outs=[x1_table.ap()[cchunk * 4096:
                                                (cchunk + 1) * 4096, :].opt()])

            layer(0)
            layer(1)

    nc.compile()
    return nc


def _prep_weights(inputs):
    out = {}
    bnb_cols = np.zeros((P, 32), np.float32)
    for ll in range(2):
        pre = "l0" if ll == 0 else "l1"
        for l in range(2):
            w1 = np.asarray(inputs[f"{pre}_w1"][l], np.float32)   # [1024, 512]
            w2 = np.asarray(inputs[f"{pre}_w2"][l], np.float32)   # [512, 1024]
            g = np.asarray(inputs[f"{pre}_g"][l], np.float32)
            b = np.asarray(inputs[f"{pre}_b"][l], np.float32)
            m = np.asarray(inputs[f"{pre}_m"][l], np.float32)
            v = np.asarray(inputs[f"{pre}_v"][l], np.float32)
            we = np.asarray(inputs[f"{pre}_edge"][l], np.float32)[:, 0]
            s = g / np.sqrt(v + BN_EPS)
            bb = b - m * s
            w1s = s[:, None] * w1
            out[f"w1t_{ll}_{l}"] = _pack_lhst(w1s.T).astype(nbf)
            out[f"w2t_{ll}_{l}"] = _pack_lhst(w2.T).astype(nbf)
            out[f"we_{ll}_{l}"] = np.ascontiguousarray(
                np.broadcast_to(we.astype(nbf), (P, H)))
            bnb_cols[:, (ll * 2 + l) * 8:(ll * 2 + l) * 8 + 8] = \
                bb.reshape(8, P).T
            if ll == 0:
                ws = np.asarray(inputs["l0_src"][l], np.float32)
                wdm = np.asarray(inputs["l0_dst"][l], np.float32)
                wf = s[:, None] * (w1 @ wdm)                      # [1024, 256]
                out[f"wst_{l}"] = np.ascontiguousarray(
                    ws.T.reshape(2, P, H).transpose(1, 0, 2))
                out[f"xw_0_{l}"] = _pack_lhst(wf.T).astype(nbf)
            else:
                out[f"xw_1_{l}"] = _pack_lhst(w1s.T).astype(nbf)
    out["bnb_all"] = bnb_cols
    out["iota"] = np.ascontiguousarray(
        np.broadcast_to(np.arange(P, dtype=np.float32), (P, P))).astype(nbf)
    return out


def kernel(**inputs):
    x = np.asarray(inputs["x"], np.float32)
    ei = np.asarray(inputs["ei_flat"], np.int32)
    ea = np.asarray(inputs["ea_flat"], np.float32)
    lens = (int(inputs["len0"]), int(inputs["len1"]))

    T, per_core = _host_prep(ei, ea, lens)
    wshared = _prep_weights(inputs)

    if T not in _cache:
        _cache[T] = _build(T)
    nc = _cache[T]

    in_maps = []
    for c in range(NCORES):
        core = per_core[c]
        lo = c * SHARD
        xs = x[lo:lo + SHARD]
        xs_pad = np.vstack([xs, np.zeros((1, FIN), np.float32)])
        xt2 = np.ascontiguousarray(xs_pad[core["perm"]].T)        # [256, SLOTS]
        xt_slt = np.ascontiguousarray(
            xt2.reshape(2, P, SLOTS).transpose(1, 0, 2))
        im = dict(xt_sf=xt_slt, xt_bf=xt_slt.astype(nbf))
        for l in range(2):
            im[f"isf_{l}"] = core[f"isf_{l}"]
            im[f"ix1_{l}"] = core[f"ix1_{l}"]
            im[f"ea_{l}"] = core[f"ea_{l}"]
            im[f"eslot_{l}"] = core[f"eslot_{l}"]
        im.update(wshared)
        in_maps.append(im)

    res = run_bass_kernel_spmd(nc, in_maps, core_ids=list(range(NCORES)))
    globals()["LAST_RESULT"] = res
    out = np.empty((N_NODES, H), np.float32)
    for c in range(NCORES):
        y = res.results[c]["y_out"].astype(np.float32)   # [4, 128, SLOTS]
        y_slots = y.reshape(H, SLOTS).T                  # [SLOTS, H]
        out[c * SHARD:(c + 1) * SHARD] = y_slots[per_core[c]["slot_of_node"]]
    return np.ascontiguousarray(out)
